# revision 15
# baseline (speedup 1.0000x reference)
"""Trainium2 Bass kernel for nn_Attention_layer_67877663146058.

Computes attn = softmax((x @ W_qkv.T)[q] @ (x @ W_qkv.T)[k]^T * hd**-0.5)
for x [8, 1024, 768], W_qkv [2304, 768] -> out [8, 12, 1024, 1024] fp32.

Sharding: batch-parallel across the 8 NeuronCores (core b handles batch b,
all 12 heads). Only the Q and K rows of W_qkv are used.

v3, shaped by measured engine rates (see git history for the derivation):
  - All HBM traffic is 16-bit: fp16 inputs/Q/K, fp16 output upconverted on
    the host. (The fp32 baseline was DMA-bound at ~150us of output writes.)
  - ACT (the only exp engine, 1.2 GHz, ~470ns/instr overhead) is the
    roofline: 48 x [128,2048] exp instructions ~= 105us. PSUM (8 banks)
    holds one shared ring of two 4-bank [128,2048] fp32 tiles used by BOTH
    the projection accumulators and the score tiles.
  - Row sums use one grouped tensor_reduce per [128, 4x1024] exp tile --
    measured ~4x fp16 rate on hardware (the static cost model claims 1x;
    the RTL auto-packs 2-byte SBUF operands).
  - tensor_scalar muls run at 4x fp16; projection PSUM->SBUF copies are
    1x (fp32 source) on DVE.
  - dma_start costs ~1.9us of SP sequencer each, so DMAs are batched:
    one [128, 4 heads x 1024] output DMA per exp tile (DRAM-side
    strided AP; the SBUF side stays plain partition-major), 5 input DMAs.
"""

import numpy as np
from contextlib import ExitStack

import concourse.bacc as bacc
import concourse.mybir as mybir
import concourse.tile as tile
from concourse.alu_op_type import AluOpType

# bass_utils imports antenv.axon_hooks when BASS_TRACE is set in the
# environment; some images ship an antenv stub without that module. Register
# a no-op fallback so tracing degrades gracefully instead of crashing.
try:
    from antenv.axon_hooks import get_axon_ntff_profile_hook as _g  # noqa: F401
except Exception:
    import sys as _sys
    import types as _types

    _m = _types.ModuleType("antenv.axon_hooks")
    _state = {"h": None}
    _m.set_axon_ntff_profile_hook = lambda h: _state.__setitem__("h", h)
    _m.get_axon_ntff_profile_hook = lambda: _state["h"]
    _sys.modules["antenv.axon_hooks"] = _m
    try:
        import antenv as _antenv

        _antenv.axon_hooks = _m
    except Exception:
        pass

from concourse.bass_utils import run_bass_kernel_spmd

B = 8          # batches == cores
N = 1024       # tokens
E = 768        # embed dim
H = 12         # heads
HD = 64        # head dim
F = H * HD     # 768 features per projection (Q or K)
ET = E // 128  # 6 e-tiles
FT = F // 128  # 6 f-tiles (2 heads per f-tile)
QB = N // 128  # 8 query blocks
G = 3          # head-quad groups (4 heads per output tile)
SCALE = HD ** -0.5

# One output DMA per exp tile (DRAM-side strided AP). Set False to fall
# back to one DMA per head (4x the SP issue cost) if the strided AP
# misbehaves.
FUSED_OUT_DMA = True
# Fraction of normalize muls offloaded to the otherwise-idle gpsimd
# engine (it runs fp16 tensor_scalar at ~2.1us/[128,1024] block vs DVE's
# ~0.6us, but DVE is the busiest engine): mul index idx=et*4+j goes to
# gpsimd when idx % 12 < POOL_MUL_NUM.
POOL_MUL_NUM = 5

_cache = {}


def _build():
    f32 = mybir.dt.float32
    f16 = mybir.dt.float16
    nc = bacc.Bacc("TRN2", debug=False, num_devices=B)

    xT_d = nc.dram_tensor("xT", [E, N], f16, kind="ExternalInput")
    wT_d = nc.dram_tensor("wT", [E, 2 * F], f16, kind="ExternalInput")
    out_d = nc.dram_tensor("out", [H, N, N], f16, kind="ExternalOutput")

    xT_src = xT_d.ap().rearrange("(t p) n -> t p n", p=128)       # [6,128,1024]
    wT_src = wT_d.ap().rearrange("(t p) f -> t p f", p=128)       # [6,128,1536]
    out_ap = out_d.ap()                                           # [12,1024,1024]

    with ExitStack() as ctx:
        tc = ctx.enter_context(tile.TileContext(nc))
        statics = ctx.enter_context(tc.tile_pool(name="statics", bufs=1))
        work = ctx.enter_context(tc.tile_pool(name="work", bufs=4))
        small = ctx.enter_context(tc.tile_pool(name="small", bufs=8))
        ring = ctx.enter_context(tc.tile_pool(name="ring", bufs=2, space="PSUM"))

        xt = statics.tile([128, ET, N], f16, tag="xt", name="xt")
        wt = statics.tile([128, ET, 2 * F], f16, tag="wt", name="wt")
        # Q/K interleaved per f-tile: index 2*fi = Q[fi], 2*fi+1 = K[fi].
        qkt = statics.tile([128, 2 * FT, N], f16, tag="qkt", name="qkt")

        # Preload the exp table set while input DMAs run: a dependency-free
        # dummy ACTIVATE at t=0 pulls the ~2.7us ACT_TABLE_LOAD off the
        # critical path of the first real exp.
        warm = small.tile([128, 1], f32, tag="warm", name="warm")
        nc.vector.memset(warm, 0.0)
        nc.scalar.activation(warm, warm, mybir.ActivationFunctionType.Exp)

        # Input loads, batched (dma_start issue cost dominates): x halves
        # interleaved with the W column chunks in dependency order.
        # Inputs issue from the ACT queue (idle until the first exp),
        # outputs from the gpsimd queue — keeps the heavily-loaded SP
        # sequencer out of the DMA path.
        nc.scalar.dma_start(xt[:, 0:3, :], xT_src[0:3].rearrange("t p n -> p t n"))
        nc.scalar.dma_start(wt[:, :, 0:512],
                            wT_src[:, :, 0:512].rearrange("t p c -> p t c"))
        nc.scalar.dma_start(xt[:, 3:6, :], xT_src[3:6].rearrange("t p n -> p t n"))
        nc.scalar.dma_start(wt[:, :, 512:1024],
                            wT_src[:, :, 512:1024].rearrange("t p c -> p t c"))
        nc.scalar.dma_start(wt[:, :, 1024:1536],
                            wT_src[:, :, 1024:1536].rearrange("t p c -> p t c"))

        def emit_proj(fi):
            # One ring tile: cols 0:1024 = Q[fi] (two 512 n-halves),
            # 1024:2048 = K[fi]. 24 accumulating matmuls, one DVE copy to
            # the fp16 qkt tile.
            pt = ring.tile([128, 2048], f32, tag="ps", name=f"pp{fi}")
            for qk in range(2):
                foff = (2 * fi + qk) * 128
                for nh in range(2):
                    col = qk * 1024 + nh * 512
                    for ei in range(ET):
                        nc.tensor.matmul(
                            pt[:, col:col + 512],
                            lhsT=wt[:, ei, foff:foff + 128],
                            rhs=xt[:, ei, nh * 512:(nh + 1) * 512],
                            start=(ei == 0),
                            stop=(ei == ET - 1),
                        )
            # The first two copies land in ACT's idle window (no exps yet);
            # the rest stay on DVE (gpsimd cannot read PSUM).
            dst = qkt[:, 2 * fi:2 * fi + 2, :].rearrange("p a n -> p (a n)")
            if fi < 2:
                nc.scalar.copy(dst, pt)
            else:
                nc.vector.tensor_copy(dst, pt)

        def emit_attn(g):
            # scores + softmax for heads 4g..4g+3 (f-tiles 2g, 2g+1), all
            # 8 q-blocks. Two ring tiles per q-block (one per f-tile, two
            # heads each), one [128,4096] fp16 exp tile, one grouped
            # reduce, one reciprocal, four muls, one output DMA.
            for qb in range(QB):
                q0, q1 = qb * 128, (qb + 1) * 128
                et = work.tile([128, 4096], f16, tag="et", name=f"et{g}_{qb}")
                for half in range(2):
                    fi = 2 * g + half
                    ps = ring.tile([128, 2048], f32, tag="ps",
                                   name=f"ps{g}_{qb}_{half}")
                    for hh in range(2):
                        lo = 64 * hh
                        for kh in range(2):
                            nc.tensor.matmul(
                                ps[:, hh * 1024 + kh * 512:
                                   hh * 1024 + kh * 512 + 512],
                                lhsT=qkt[lo:lo + 64, 2 * fi, q0:q1],
                                rhs=qkt[lo:lo + 64, 2 * fi + 1,
                                        kh * 512:kh * 512 + 512],
                                start=True,
                                stop=True,
                                tile_position=(lo, 0),
                            )
                    nc.scalar.activation(
                        et[:, half * 2048:(half + 1) * 2048], ps,
                        mybir.ActivationFunctionType.Exp, scale=SCALE,
                    )
                # Per-head reduce-to-scalar hits the DVE fast path (~4x);
                # grouped reduces with multi-element outputs run 1x.
                sums = small.tile([128, 4], f16, tag="sums", name=f"sm{g}_{qb}")
                with nc.allow_low_precision(reason="fp16 row sums of positive exp values; ulp 2^-11 of ~1700 is ~0.05%"):
                    for j in range(4):
                        nc.vector.tensor_reduce(
                            sums[:, j:j + 1], et[:, j * N:(j + 1) * N],
                            axis=mybir.AxisListType.X, op=AluOpType.add,
                        )
                rec = small.tile([128, 4], f32, tag="rec", name=f"rc{g}_{qb}")
                nc.vector.reciprocal(rec, sums)
                for j in range(4):
                    idx = (g * QB + qb) * 4 + j
                    eng = nc.gpsimd if idx % 12 < POOL_MUL_NUM else nc.vector
                    eng.tensor_scalar_mul(
                        et[:, j * N:(j + 1) * N],
                        et[:, j * N:(j + 1) * N],
                        rec[:, j:j + 1],
                    )
                if FUSED_OUT_DMA:
                    nc.gpsimd.dma_start(
                        out_ap[4 * g:4 * g + 4, q0:q1, :]
                        .rearrange("h q n -> q h n"),
                        et,
                    )
                else:
                    for j in range(4):
                        nc.gpsimd.dma_start(
                            out_ap[4 * g + j, q0:q1, :],
                            et[:, j * N:(j + 1) * N],
                        )

        # Interleave projections with score groups: scores for group g need
        # projections 2g and 2g+1; later projections fill PE gaps while ACT
        # drains the current group's score tiles.
        emit_proj(0)
        emit_proj(1)
        emit_attn(0)          # heads 0-3
        emit_proj(2)
        emit_proj(3)
        emit_attn(1)          # heads 4-7
        emit_proj(4)
        emit_proj(5)
        emit_attn(2)          # heads 8-11

    nc.compile()
    return nc


def _run(x, W_qkv, trace=False):
    if "nc" not in _cache:
        _cache["nc"] = _build()
    nc = _cache["nc"]

    x = np.asarray(x, dtype=np.float32)
    W_qkv = np.asarray(W_qkv, dtype=np.float32)
    # interleave Q/K 128-col blocks per f-tile: [Q0,K0,Q1,K1,...,Q5,K5]
    wqk = W_qkv[: 2 * F].reshape(2, FT, 128, E)           # [qk, fi, 128, e]
    wqk = wqk.transpose(3, 1, 0, 2).reshape(E, 2 * F)     # [e, fi*qk*128]
    wT = np.ascontiguousarray(wqk.astype(np.float16))     # [768, 1536]
    in_maps = [
        {"xT": np.ascontiguousarray(x[b].T.astype(np.float16)), "wT": wT}
        for b in range(B)
    ]
    res = run_bass_kernel_spmd(nc, in_maps, core_ids=list(range(B)), trace=trace)
    out = np.stack([np.asarray(r["out"], dtype=np.float32) for r in res.results], axis=0)
    return out, res


def kernel(x, W_qkv):
    return _run(x, W_qkv)[0]


# revision 16
# speedup vs baseline: 1.0133x; 1.0133x over previous
"""Trainium2 Bass kernel for nn_Attention_layer_67877663146058.

Computes attn = softmax((x @ W_qkv.T)[q] @ (x @ W_qkv.T)[k]^T * hd**-0.5)
for x [8, 1024, 768], W_qkv [2304, 768] -> out [8, 12, 1024, 1024] fp32.

Sharding: batch-parallel across the 8 NeuronCores (core b handles batch b,
all 12 heads). Only the Q and K rows of W_qkv are used.

v3, shaped by measured engine rates (see git history for the derivation):
  - All HBM traffic is 16-bit: fp16 inputs/Q/K, fp16 output upconverted on
    the host. (The fp32 baseline was DMA-bound at ~150us of output writes.)
  - ACT (the only exp engine, 1.2 GHz, ~470ns/instr overhead) is the
    roofline: 48 x [128,2048] exp instructions ~= 105us. PSUM (8 banks)
    holds one shared ring of two 4-bank [128,2048] fp32 tiles used by BOTH
    the projection accumulators and the score tiles.
  - Row sums use one grouped tensor_reduce per [128, 4x1024] exp tile --
    measured ~4x fp16 rate on hardware (the static cost model claims 1x;
    the RTL auto-packs 2-byte SBUF operands).
  - tensor_scalar muls run at 4x fp16; projection PSUM->SBUF copies are
    1x (fp32 source) on DVE.
  - dma_start costs ~1.9us of SP sequencer each, so DMAs are batched:
    one [128, 4 heads x 1024] output DMA per exp tile (DRAM-side
    strided AP; the SBUF side stays plain partition-major), 5 input DMAs.
"""

import numpy as np
from contextlib import ExitStack

import concourse.bacc as bacc
import concourse.mybir as mybir
import concourse.tile as tile
from concourse.alu_op_type import AluOpType

# bass_utils imports antenv.axon_hooks when BASS_TRACE is set in the
# environment; some images ship an antenv stub without that module. Register
# a no-op fallback so tracing degrades gracefully instead of crashing.
try:
    from antenv.axon_hooks import get_axon_ntff_profile_hook as _g  # noqa: F401
except Exception:
    import sys as _sys
    import types as _types

    _m = _types.ModuleType("antenv.axon_hooks")
    _state = {"h": None}
    _m.set_axon_ntff_profile_hook = lambda h: _state.__setitem__("h", h)
    _m.get_axon_ntff_profile_hook = lambda: _state["h"]
    _sys.modules["antenv.axon_hooks"] = _m
    try:
        import antenv as _antenv

        _antenv.axon_hooks = _m
    except Exception:
        pass

from concourse.bass_utils import run_bass_kernel_spmd

B = 8          # batches == cores
N = 1024       # tokens
E = 768        # embed dim
H = 12         # heads
HD = 64        # head dim
F = H * HD     # 768 features per projection (Q or K)
ET = E // 128  # 6 e-tiles
FT = F // 128  # 6 f-tiles (2 heads per f-tile)
QB = N // 128  # 8 query blocks
G = 3          # head-quad groups (4 heads per output tile)
SCALE = HD ** -0.5

# One output DMA per exp tile (DRAM-side strided AP). Set False to fall
# back to one DMA per head (4x the SP issue cost) if the strided AP
# misbehaves.
FUSED_OUT_DMA = True
# Fraction of normalize muls offloaded to the otherwise-idle gpsimd
# engine (it runs fp16 tensor_scalar at ~2.1us/[128,1024] block vs DVE's
# ~0.6us, but DVE is the busiest engine): mul index idx=et*4+j goes to
# gpsimd when idx % 12 < POOL_MUL_NUM.
POOL_MUL_NUM = 5

_cache = {}


def _build():
    f32 = mybir.dt.float32
    f16 = mybir.dt.float16
    nc = bacc.Bacc("TRN2", debug=False, num_devices=B)

    xT_d = nc.dram_tensor("xT", [E, N], f16, kind="ExternalInput")
    wT_d = nc.dram_tensor("wT", [E, 2 * F], f16, kind="ExternalInput")
    out_d = nc.dram_tensor("out", [H, N, N], f16, kind="ExternalOutput")

    xT_src = xT_d.ap().rearrange("(t p) n -> t p n", p=128)       # [6,128,1024]
    wT_src = wT_d.ap().rearrange("(t p) f -> t p f", p=128)       # [6,128,1536]
    out_ap = out_d.ap()                                           # [12,1024,1024]

    with ExitStack() as ctx:
        tc = ctx.enter_context(tile.TileContext(nc))
        statics = ctx.enter_context(tc.tile_pool(name="statics", bufs=1))
        work = ctx.enter_context(tc.tile_pool(name="work", bufs=4))
        small = ctx.enter_context(tc.tile_pool(name="small", bufs=8))
        ring = ctx.enter_context(tc.tile_pool(name="ring", bufs=2, space="PSUM"))

        xt = statics.tile([128, ET, N], f16, tag="xt", name="xt")
        wt = statics.tile([128, ET, 2 * F], f16, tag="wt", name="wt")
        # Q/K interleaved per f-tile: index 2*fi = Q[fi], 2*fi+1 = K[fi].
        qkt = statics.tile([128, 2 * FT, N], f16, tag="qkt", name="qkt")

        # Preload the exp table set while input DMAs run: a dependency-free
        # dummy ACTIVATE at t=0 pulls the ~2.7us ACT_TABLE_LOAD off the
        # critical path of the first real exp.
        warm = small.tile([128, 1], f32, tag="warm", name="warm")
        nc.vector.memset(warm, 0.0)
        nc.scalar.activation(warm, warm, mybir.ActivationFunctionType.Exp)

        # Input loads, batched (dma_start issue cost dominates): x halves
        # interleaved with the W column chunks in dependency order.
        nc.sync.dma_start(xt[:, 0:3, :], xT_src[0:3].rearrange("t p n -> p t n"))
        nc.sync.dma_start(wt[:, :, 0:512],
                            wT_src[:, :, 0:512].rearrange("t p c -> p t c"))
        nc.sync.dma_start(xt[:, 3:6, :], xT_src[3:6].rearrange("t p n -> p t n"))
        nc.sync.dma_start(wt[:, :, 512:1024],
                            wT_src[:, :, 512:1024].rearrange("t p c -> p t c"))
        nc.sync.dma_start(wt[:, :, 1024:1536],
                            wT_src[:, :, 1024:1536].rearrange("t p c -> p t c"))

        def emit_proj(fi):
            # One ring tile: cols 0:1024 = Q[fi] (two 512 n-halves),
            # 1024:2048 = K[fi]. 24 accumulating matmuls, one DVE copy to
            # the fp16 qkt tile.
            pt = ring.tile([128, 2048], f32, tag="ps", name=f"pp{fi}")
            for qk in range(2):
                foff = (2 * fi + qk) * 128
                for nh in range(2):
                    col = qk * 1024 + nh * 512
                    for ei in range(ET):
                        nc.tensor.matmul(
                            pt[:, col:col + 512],
                            lhsT=wt[:, ei, foff:foff + 128],
                            rhs=xt[:, ei, nh * 512:(nh + 1) * 512],
                            start=(ei == 0),
                            stop=(ei == ET - 1),
                        )
            # The first two copies land in ACT's idle window (no exps yet);
            # the rest stay on DVE (gpsimd cannot read PSUM).
            dst = qkt[:, 2 * fi:2 * fi + 2, :].rearrange("p a n -> p (a n)")
            if fi < 2:
                nc.scalar.copy(dst, pt)
            else:
                nc.vector.tensor_copy(dst, pt)

        def emit_attn(g):
            # scores + softmax for heads 4g..4g+3 (f-tiles 2g, 2g+1), all
            # 8 q-blocks. Two ring tiles per q-block (one per f-tile, two
            # heads each), one [128,4096] fp16 exp tile, one grouped
            # reduce, one reciprocal, four muls, one output DMA.
            for qb in range(QB):
                q0, q1 = qb * 128, (qb + 1) * 128
                et = work.tile([128, 4096], f16, tag="et", name=f"et{g}_{qb}")
                for half in range(2):
                    fi = 2 * g + half
                    ps = ring.tile([128, 2048], f32, tag="ps",
                                   name=f"ps{g}_{qb}_{half}")
                    for hh in range(2):
                        lo = 64 * hh
                        for kh in range(2):
                            nc.tensor.matmul(
                                ps[:, hh * 1024 + kh * 512:
                                   hh * 1024 + kh * 512 + 512],
                                lhsT=qkt[lo:lo + 64, 2 * fi, q0:q1],
                                rhs=qkt[lo:lo + 64, 2 * fi + 1,
                                        kh * 512:kh * 512 + 512],
                                start=True,
                                stop=True,
                                tile_position=(lo, 0),
                            )
                    nc.scalar.activation(
                        et[:, half * 2048:(half + 1) * 2048], ps,
                        mybir.ActivationFunctionType.Exp, scale=SCALE,
                    )
                # Per-head reduce-to-scalar hits the DVE fast path (~4x);
                # grouped reduces with multi-element outputs run 1x.
                sums = small.tile([128, 4], f16, tag="sums", name=f"sm{g}_{qb}")
                with nc.allow_low_precision(reason="fp16 row sums of positive exp values; ulp 2^-11 of ~1700 is ~0.05%"):
                    for j in range(4):
                        nc.vector.tensor_reduce(
                            sums[:, j:j + 1], et[:, j * N:(j + 1) * N],
                            axis=mybir.AxisListType.X, op=AluOpType.add,
                        )
                rec = small.tile([128, 4], f32, tag="rec", name=f"rc{g}_{qb}")
                nc.vector.reciprocal(rec, sums)
                for j in range(4):
                    idx = (g * QB + qb) * 4 + j
                    eng = nc.gpsimd if idx % 12 < POOL_MUL_NUM else nc.vector
                    eng.tensor_scalar_mul(
                        et[:, j * N:(j + 1) * N],
                        et[:, j * N:(j + 1) * N],
                        rec[:, j:j + 1],
                    )
                if FUSED_OUT_DMA:
                    nc.sync.dma_start(
                        out_ap[4 * g:4 * g + 4, q0:q1, :]
                        .rearrange("h q n -> q h n"),
                        et,
                    )
                else:
                    for j in range(4):
                        nc.sync.dma_start(
                            out_ap[4 * g + j, q0:q1, :],
                            et[:, j * N:(j + 1) * N],
                        )

        # Interleave projections with score groups: scores for group g need
        # projections 2g and 2g+1; later projections fill PE gaps while ACT
        # drains the current group's score tiles.
        emit_proj(0)
        emit_proj(1)
        emit_attn(0)          # heads 0-3
        emit_proj(2)
        emit_proj(3)
        emit_attn(1)          # heads 4-7
        emit_proj(4)
        emit_proj(5)
        emit_attn(2)          # heads 8-11

    nc.compile()
    return nc


def _run(x, W_qkv, trace=False):
    if "nc" not in _cache:
        _cache["nc"] = _build()
    nc = _cache["nc"]

    x = np.asarray(x, dtype=np.float32)
    W_qkv = np.asarray(W_qkv, dtype=np.float32)
    # interleave Q/K 128-col blocks per f-tile: [Q0,K0,Q1,K1,...,Q5,K5]
    wqk = W_qkv[: 2 * F].reshape(2, FT, 128, E)           # [qk, fi, 128, e]
    wqk = wqk.transpose(3, 1, 0, 2).reshape(E, 2 * F)     # [e, fi*qk*128]
    wT = np.ascontiguousarray(wqk.astype(np.float16))     # [768, 1536]
    in_maps = [
        {"xT": np.ascontiguousarray(x[b].T.astype(np.float16)), "wT": wT}
        for b in range(B)
    ]
    res = run_bass_kernel_spmd(nc, in_maps, core_ids=list(range(B)), trace=trace)
    out = np.stack([np.asarray(r["out"], dtype=np.float32) for r in res.results], axis=0)
    return out, res


def kernel(x, W_qkv):
    return _run(x, W_qkv)[0]


# revision 17
# speedup vs baseline: 2.9700x; 2.9311x over previous
"""Trainium2 Bass kernel for nn_Attention_layer_67877663146058.

Computes attn = softmax((x @ W_qkv.T)[q] @ (x @ W_qkv.T)[k]^T * hd**-0.5)
for x [8, 1024, 768], W_qkv [2304, 768] -> out [8, 12, 1024, 1024] fp32.

Sharding: batch-parallel across the 8 NeuronCores (core b handles batch b,
all 12 heads). Only the Q and K rows of W_qkv are used.

v3, shaped by measured engine rates (see git history for the derivation):
  - All HBM traffic is 16-bit: fp16 inputs/Q/K, fp16 output upconverted on
    the host. (The fp32 baseline was DMA-bound at ~150us of output writes.)
  - ACT (the only exp engine, 1.2 GHz, ~470ns/instr overhead) is the
    roofline: 48 x [128,2048] exp instructions ~= 105us. PSUM (8 banks)
    holds one shared ring of two 4-bank [128,2048] fp32 tiles used by BOTH
    the projection accumulators and the score tiles.
  - Row sums use one grouped tensor_reduce per [128, 4x1024] exp tile --
    measured ~4x fp16 rate on hardware (the static cost model claims 1x;
    the RTL auto-packs 2-byte SBUF operands).
  - tensor_scalar muls run at 4x fp16; projection PSUM->SBUF copies are
    1x (fp32 source) on DVE.
  - dma_start costs ~1.9us of SP sequencer each, so DMAs are batched:
    one [128, 4 heads x 1024] output DMA per exp tile (DRAM-side
    strided AP; the SBUF side stays plain partition-major), 5 input DMAs.
"""

import numpy as np
from contextlib import ExitStack

import concourse.bacc as bacc
import concourse.mybir as mybir
import concourse.tile as tile
from concourse.alu_op_type import AluOpType

# bass_utils imports antenv.axon_hooks when BASS_TRACE is set in the
# environment; some images ship an antenv stub without that module. Register
# a no-op fallback so tracing degrades gracefully instead of crashing.
try:
    from antenv.axon_hooks import get_axon_ntff_profile_hook as _g  # noqa: F401
except Exception:
    import sys as _sys
    import types as _types

    _m = _types.ModuleType("antenv.axon_hooks")
    _state = {"h": None}
    _m.set_axon_ntff_profile_hook = lambda h: _state.__setitem__("h", h)
    _m.get_axon_ntff_profile_hook = lambda: _state["h"]
    _sys.modules["antenv.axon_hooks"] = _m
    try:
        import antenv as _antenv

        _antenv.axon_hooks = _m
    except Exception:
        pass

from concourse.bass_utils import run_bass_kernel_spmd

B = 8          # batches == cores
N = 1024       # tokens
E = 768        # embed dim
H = 12         # heads
HD = 64        # head dim
F = H * HD     # 768 features per projection (Q or K)
ET = E // 128  # 6 e-tiles
FT = F // 128  # 6 f-tiles (2 heads per f-tile)
QB = N // 128  # 8 query blocks
G = 3          # head-quad groups (4 heads per output tile)
SCALE = HD ** -0.5

# One output DMA per exp tile (DRAM-side strided AP). Set False to fall
# back to one DMA per head (4x the SP issue cost) if the strided AP
# misbehaves.
FUSED_OUT_DMA = True
# Fraction of normalize muls offloaded to the otherwise-idle gpsimd
# engine (it runs fp16 tensor_scalar at ~2.1us/[128,1024] block vs DVE's
# ~0.6us, but DVE is the busiest engine): mul index idx=et*4+j goes to
# gpsimd when idx % 12 < POOL_MUL_NUM.
POOL_MUL_NUM = 0

_cache = {}


def _build():
    f32 = mybir.dt.float32
    f16 = mybir.dt.float16
    nc = bacc.Bacc("TRN2", debug=False, num_devices=B)

    xT_d = nc.dram_tensor("xT", [E, N], f16, kind="ExternalInput")
    wT_d = nc.dram_tensor("wT", [E, 2 * F], f16, kind="ExternalInput")
    out_d = nc.dram_tensor("out", [H, N, N], f16, kind="ExternalOutput")

    xT_src = xT_d.ap().rearrange("(t p) n -> t p n", p=128)       # [6,128,1024]
    wT_src = wT_d.ap().rearrange("(t p) f -> t p f", p=128)       # [6,128,1536]
    out_ap = out_d.ap()                                           # [12,1024,1024]

    with ExitStack() as ctx:
        tc = ctx.enter_context(tile.TileContext(nc))
        statics = ctx.enter_context(tc.tile_pool(name="statics", bufs=1))
        work = ctx.enter_context(tc.tile_pool(name="work", bufs=4))
        small = ctx.enter_context(tc.tile_pool(name="small", bufs=8))
        ring = ctx.enter_context(tc.tile_pool(name="ring", bufs=2, space="PSUM"))

        xt = statics.tile([128, ET, N], f16, tag="xt", name="xt")
        wt = statics.tile([128, ET, 2 * F], f16, tag="wt", name="wt")
        # Q/K interleaved per f-tile: index 2*fi = Q[fi], 2*fi+1 = K[fi].
        qkt = statics.tile([128, 2 * FT, N], f16, tag="qkt", name="qkt")

        # Preload the exp table set while input DMAs run: a dependency-free
        # dummy ACTIVATE at t=0 pulls the ~2.7us ACT_TABLE_LOAD off the
        # critical path of the first real exp.
        warm = small.tile([128, 1], f32, tag="warm", name="warm")
        nc.vector.memset(warm, 0.0)
        nc.scalar.activation(warm, warm, mybir.ActivationFunctionType.Exp)

        # Input loads, batched (dma_start issue cost dominates): x halves
        # interleaved with the W column chunks in dependency order.
        nc.sync.dma_start(xt[:, 0:3, :], xT_src[0:3].rearrange("t p n -> p t n"))
        nc.sync.dma_start(wt[:, :, 0:512],
                            wT_src[:, :, 0:512].rearrange("t p c -> p t c"))
        nc.sync.dma_start(xt[:, 3:6, :], xT_src[3:6].rearrange("t p n -> p t n"))
        nc.sync.dma_start(wt[:, :, 512:1024],
                            wT_src[:, :, 512:1024].rearrange("t p c -> p t c"))
        nc.sync.dma_start(wt[:, :, 1024:1536],
                            wT_src[:, :, 1024:1536].rearrange("t p c -> p t c"))

        def emit_proj(fi):
            # One ring tile: cols 0:1024 = Q[fi] (two 512 n-halves),
            # 1024:2048 = K[fi]. 24 accumulating matmuls, one DVE copy to
            # the fp16 qkt tile.
            pt = ring.tile([128, 2048], f32, tag="ps", name=f"pp{fi}")
            for qk in range(2):
                foff = (2 * fi + qk) * 128
                for nh in range(2):
                    col = qk * 1024 + nh * 512
                    for ei in range(ET):
                        nc.tensor.matmul(
                            pt[:, col:col + 512],
                            lhsT=wt[:, ei, foff:foff + 128],
                            rhs=xt[:, ei, nh * 512:(nh + 1) * 512],
                            start=(ei == 0),
                            stop=(ei == ET - 1),
                        )
            # The first two copies land in ACT's idle window (no exps yet);
            # the rest stay on DVE (gpsimd cannot read PSUM).
            dst = qkt[:, 2 * fi:2 * fi + 2, :].rearrange("p a n -> p (a n)")
            if fi < 2:
                nc.scalar.copy(dst, pt)
            else:
                nc.vector.tensor_copy(dst, pt)

        def emit_attn(g):
            # scores + softmax for heads 4g..4g+3 (f-tiles 2g, 2g+1), all
            # 8 q-blocks. Two ring tiles per q-block (one per f-tile, two
            # heads each), one [128,4096] fp16 exp tile, one grouped
            # reduce, one reciprocal, four muls, one output DMA.
            for qb in range(QB):
                q0, q1 = qb * 128, (qb + 1) * 128
                et = work.tile([128, 4096], f16, tag="et", name=f"et{g}_{qb}")
                for half in range(2):
                    fi = 2 * g + half
                    ps = ring.tile([128, 2048], f32, tag="ps",
                                   name=f"ps{g}_{qb}_{half}")
                    for hh in range(2):
                        lo = 64 * hh
                        for kh in range(2):
                            nc.tensor.matmul(
                                ps[:, hh * 1024 + kh * 512:
                                   hh * 1024 + kh * 512 + 512],
                                lhsT=qkt[lo:lo + 64, 2 * fi, q0:q1],
                                rhs=qkt[lo:lo + 64, 2 * fi + 1,
                                        kh * 512:kh * 512 + 512],
                                start=True,
                                stop=True,
                                tile_position=(lo, 0),
                            )
                    nc.scalar.activation(
                        et[:, half * 2048:(half + 1) * 2048], ps,
                        mybir.ActivationFunctionType.Exp, scale=SCALE,
                    )
                # Per-head reduce-to-scalar hits the DVE fast path (~4x);
                # grouped reduces with multi-element outputs run 1x.
                sums = small.tile([128, 4], f16, tag="sums", name=f"sm{g}_{qb}")
                with nc.allow_low_precision(reason="fp16 row sums of positive exp values; ulp 2^-11 of ~1700 is ~0.05%"):
                    for j in range(4):
                        nc.vector.tensor_reduce(
                            sums[:, j:j + 1], et[:, j * N:(j + 1) * N],
                            axis=mybir.AxisListType.X, op=AluOpType.add,
                        )
                rec = small.tile([128, 4], f32, tag="rec", name=f"rc{g}_{qb}")
                nc.vector.reciprocal(rec, sums)
                for j in range(4):
                    idx = (g * QB + qb) * 4 + j
                    eng = nc.gpsimd if idx % 12 < POOL_MUL_NUM else nc.vector
                    eng.tensor_scalar_mul(
                        et[:, j * N:(j + 1) * N],
                        et[:, j * N:(j + 1) * N],
                        rec[:, j:j + 1],
                    )
                if FUSED_OUT_DMA:
                    nc.sync.dma_start(
                        out_ap[4 * g:4 * g + 4, q0:q1, :]
                        .rearrange("h q n -> q h n"),
                        et,
                    )
                else:
                    for j in range(4):
                        nc.sync.dma_start(
                            out_ap[4 * g + j, q0:q1, :],
                            et[:, j * N:(j + 1) * N],
                        )

        # Interleave projections with score groups: scores for group g need
        # projections 2g and 2g+1; later projections fill PE gaps while ACT
        # drains the current group's score tiles.
        emit_proj(0)
        emit_proj(1)
        emit_attn(0)          # heads 0-3
        emit_proj(2)
        emit_proj(3)
        emit_attn(1)          # heads 4-7
        emit_proj(4)
        emit_proj(5)
        emit_attn(2)          # heads 8-11

    nc.compile()
    return nc


def _run(x, W_qkv, trace=False):
    if "nc" not in _cache:
        _cache["nc"] = _build()
    nc = _cache["nc"]

    x = np.asarray(x, dtype=np.float32)
    W_qkv = np.asarray(W_qkv, dtype=np.float32)
    # interleave Q/K 128-col blocks per f-tile: [Q0,K0,Q1,K1,...,Q5,K5]
    wqk = W_qkv[: 2 * F].reshape(2, FT, 128, E)           # [qk, fi, 128, e]
    wqk = wqk.transpose(3, 1, 0, 2).reshape(E, 2 * F)     # [e, fi*qk*128]
    wT = np.ascontiguousarray(wqk.astype(np.float16))     # [768, 1536]
    in_maps = [
        {"xT": np.ascontiguousarray(x[b].T.astype(np.float16)), "wT": wT}
        for b in range(B)
    ]
    res = run_bass_kernel_spmd(nc, in_maps, core_ids=list(range(B)), trace=trace)
    out = np.stack([np.asarray(r["out"], dtype=np.float32) for r in res.results], axis=0)
    return out, res


def kernel(x, W_qkv):
    return _run(x, W_qkv)[0]


# revision 18
# speedup vs baseline: 3.8755x; 1.3049x over previous
"""Trainium2 Bass kernel for nn_Attention_layer_67877663146058.

Computes attn = softmax((x @ W_qkv.T)[q] @ (x @ W_qkv.T)[k]^T * hd**-0.5)
for x [8, 1024, 768], W_qkv [2304, 768] -> out [8, 12, 1024, 1024] fp32.

Sharding: batch-parallel across the 8 NeuronCores (core b handles batch b,
all 12 heads). The V third of the QKV projection never reaches the output,
so only the Q and K rows of W_qkv are used.

Layout strategy: the PE contracts over the partition dim of both operands,
so the projection needs x^T [e, n] and W^T [e, f] — both produced on the
host (cheap numpy transposes during input prep; DMA transpose on TRN2 is
2-byte-dtype-only). The projection output Q^T/K^T [f, n] is then exactly
the [d, n] layout the scores matmul wants for both operands.

Matmuls run as float32r (same fp32 bytes, faster PE mode: 1 cycle/row vs
2-4 for plain fp32). The two heads that share an f-tile occupy PE row
groups 0:64 / 64:128 via tile_position so their K=64 score matmuls overlap.

Softmax skips the max-subtraction (scores are ~N(0,1) after the 1/8 scale;
exp never overflows fp32) so the only per-element passes are:
  PE matmul -> PSUM, ACT exp (+free row-sum accumulator) -> SBUF,
  DVE per-row scale -> SBUF, DMA -> HBM.
"""

import numpy as np
from contextlib import ExitStack

import concourse.bacc as bacc
import concourse.mybir as mybir
import concourse.tile as tile

# bass_utils imports antenv.axon_hooks when BASS_TRACE is set in the
# environment; some images ship an antenv stub without that module. Register
# a no-op fallback so tracing degrades gracefully instead of crashing.
try:
    from antenv.axon_hooks import get_axon_ntff_profile_hook as _g  # noqa: F401
except Exception:
    import sys as _sys
    import types as _types

    _m = _types.ModuleType("antenv.axon_hooks")
    _state = {"h": None}
    _m.set_axon_ntff_profile_hook = lambda h: _state.__setitem__("h", h)
    _m.get_axon_ntff_profile_hook = lambda: _state["h"]
    _sys.modules["antenv.axon_hooks"] = _m
    try:
        import antenv as _antenv

        _antenv.axon_hooks = _m
    except Exception:
        pass

from concourse.bass_utils import run_bass_kernel_spmd

B = 8          # batches == cores
N = 1024       # tokens
E = 768        # embed dim
H = 12         # heads
HD = 64        # head dim
F = H * HD     # 768 features per projection (Q or K)
ET = E // 128  # 6 e-tiles
FT = F // 128  # 6 f-tiles (2 heads per f-tile)
QB = N // 128  # 8 query blocks
SCALE = HD ** -0.5

_cache = {}


def _build(use_f32r=True):
    f32 = mybir.dt.float32
    mm_dt = mybir.dt.float32r if use_f32r else f32
    nc = bacc.Bacc("TRN2", debug=False, num_devices=B)

    xT_d = nc.dram_tensor("xT", [E, N], f32, kind="ExternalInput")
    wT_d = nc.dram_tensor("wT", [E, 2 * F], f32, kind="ExternalInput")
    out_d = nc.dram_tensor("out", [H, N, N], f32, kind="ExternalOutput")

    xT_src = xT_d.ap().rearrange("(t p) n -> t p n", p=128)       # [6,128,1024]
    wT_src = wT_d.ap().rearrange("(t p) f -> t p f", p=128)       # [6,128,1536]
    out_flat = out_d.ap().rearrange("h q n -> (h q) n")           # [12288,1024]

    def mm(out_ap, lhsT, rhs, **kw):
        nc.tensor.matmul(out_ap, lhsT, rhs, **kw)

    with ExitStack() as ctx:
        tc = ctx.enter_context(tile.TileContext(nc))
        statics = ctx.enter_context(tc.tile_pool(name="statics", bufs=1))
        work = ctx.enter_context(tc.tile_pool(name="work", bufs=8))
        small = ctx.enter_context(tc.tile_pool(name="small", bufs=8))
        pproj = ctx.enter_context(tc.tile_pool(name="pproj", bufs=2, space="PSUM"))
        pscore = ctx.enter_context(tc.tile_pool(name="pscore", bufs=3, space="PSUM"))

        xt = statics.tile([128, ET, N], mm_dt, tag="xt", name="xt")
        wt = statics.tile([128, ET, 2 * F], mm_dt, tag="wt", name="wt")
        qt = statics.tile([128, FT, N], mm_dt, tag="qt", name="qt")
        kt = statics.tile([128, FT, N], mm_dt, tag="kt", name="kt")

        # Preload the exp table set while input DMAs run: a dependency-free
        # dummy ACTIVATE at t=0 pulls the ~2.7us ACT_TABLE_LOAD off the
        # critical path of the first real exp.
        warm = small.tile([128, 1], f32, tag="sums", name="warm")
        nc.vector.memset(warm, 0.0)
        nc.scalar.activation(warm, warm, mybir.ActivationFunctionType.Exp)

        # Input loads, chunked per e-tile so the first projection matmuls can
        # start as soon as the first chunks land.
        # Single sync-ring FIFO, priority-ordered: x chunks and the W columns
        # for f-tiles 0-1 first (they gate projections 0-1), then the rest in
        # f-tile order. 512-col chunks keep DMA descriptor runs at 2KB.
        for ei in range(ET):
            nc.sync.dma_start(xt[:, ei, :], xT_src[ei].bitcast(mm_dt))
            nc.sync.dma_start(wt[:, ei, 0:256], wT_src[ei][:, 0:256].bitcast(mm_dt))
        for ei in range(ET):
            nc.sync.dma_start(wt[:, ei, 256:512], wT_src[ei][:, 256:512].bitcast(mm_dt))
        for fg in range(1, 3):
            c0, c1 = fg * 512, (fg + 1) * 512
            for ei in range(ET):
                nc.sync.dma_start(
                    wt[:, ei, c0:c1], wT_src[ei][:, c0:c1].bitcast(mm_dt)
                )

        def emit_proj(fi):
            # qT/kT tile fi = W^T-cols.T @ x^T, as four single-bank [128,512]
            # accumulation tiles so projection holds only 2 PSUM banks
            # (bufs=2 keeps copy-read and next-group matmul-write in
            # disjoint banks), freeing banks for deeper scores buffering.
            # K halves first: kt gates every scores rhs.
            for dst, foff, nh in (
                (kt, (2 * fi + 1) * 128, 0),
                (kt, (2 * fi + 1) * 128, 1),
                (qt, 2 * fi * 128, 0),
                (qt, 2 * fi * 128, 1),
            ):
                pt = pproj.tile([128, 512], f32, tag="proj",
                                name=f"pp{fi}_{foff}_{nh}")
                for ei in range(ET):
                    mm(
                        pt,
                        lhsT=wt[:, ei, foff:foff + 128],
                        rhs=xt[:, ei, nh * 512:(nh + 1) * 512],
                        start=(ei == 0),
                        stop=(ei == ET - 1),
                    )
                nc.vector.tensor_copy(dst[:, fi, nh * 512:(nh + 1) * 512], pt)

        def emit_attn(fi):
            # scores + softmax for the two heads in this f-tile. Head 2fi
            # lives in partitions 0:64, head 2fi+1 in 64:128 -> their K=64
            # matmuls target different PE row groups and run concurrently.
            for qb in range(QB):
                scores = [
                    pscore.tile([128, N], f32, tag="ps", name=f"ps{fi}_{qb}_{hh}")
                    for hh in range(2)
                ]
                for hh in range(2):
                    for nh in range(2):
                        lo, hi = hh * 64, hh * 64 + 64
                        mm(
                            scores[hh][:, nh * 512:(nh + 1) * 512],
                            lhsT=qt[lo:hi, fi, qb * 128:(qb + 1) * 128],
                            rhs=kt[lo:hi, fi, nh * 512:(nh + 1) * 512],
                            start=True,
                            stop=True,
                            tile_position=(hh * 64, 0),
                        )
                for hh in range(2):
                    h = 2 * fi + hh
                    ot = work.tile([128, N], f32, tag="out", name=f"ot{fi}_{qb}_{hh}")
                    sums = small.tile([128, 1], f32, tag="sums", name=f"sm{fi}_{qb}_{hh}")
                    nc.scalar.activation(
                        ot, scores[hh], mybir.ActivationFunctionType.Exp,
                        scale=SCALE, accum_out=sums,
                    )
                    rec = small.tile([128, 1], f32, tag="rec", name=f"rc{fi}_{qb}_{hh}")
                    nc.vector.reciprocal(rec, sums)
                    nc.vector.tensor_scalar_mul(ot, ot, rec)
                    nc.sync.dma_start(
                        out_flat[h * N + qb * 128:h * N + (qb + 1) * 128], ot
                    )

        for fi in range(FT):
            emit_proj(fi)
            emit_attn(fi)

    nc.compile()
    return nc


def _run(x, W_qkv, trace=False, use_f32r=True):
    key = ("nc", use_f32r)
    if key not in _cache:
        _cache[key] = _build(use_f32r)
    nc = _cache[key]

    x = np.asarray(x, dtype=np.float32)
    W_qkv = np.asarray(W_qkv, dtype=np.float32)
    # interleave Q/K 128-col blocks per f-tile: [Q0,K0,Q1,K1,...,Q5,K5]
    wqk = W_qkv[: 2 * F].reshape(2, FT, 128, E)           # [qk, fi, 128, e]
    wqk = wqk.transpose(3, 1, 0, 2).reshape(E, 2 * F)     # [e, fi*qk*128]
    wT = np.ascontiguousarray(wqk)                        # [768, 1536]
    in_maps = [
        {"xT": np.ascontiguousarray(x[b].T), "wT": wT}
        for b in range(B)
    ]
    res = run_bass_kernel_spmd(nc, in_maps, core_ids=list(range(B)), trace=trace)
    out = np.stack([r["out"] for r in res.results], axis=0)
    return out, res


def kernel(x, W_qkv):
    return _run(x, W_qkv)[0]



# revision 19
# speedup vs baseline: 4.1756x; 1.0774x over previous
"""Trainium2 Bass kernel for nn_Attention_layer_67877663146058.

Computes attn = softmax((x @ W_qkv.T)[q] @ (x @ W_qkv.T)[k]^T * hd**-0.5)
for x [8, 1024, 768], W_qkv [2304, 768] -> out [8, 12, 1024, 1024] fp32.

Sharding: batch-parallel across the 8 NeuronCores (core b handles batch b,
all 12 heads). The V third of the QKV projection never reaches the output,
so only the Q and K rows of W_qkv are used.

Layout strategy: the PE contracts over the partition dim of both operands,
so the projection needs x^T [e, n] and W^T [e, f] — both produced on the
host (cheap numpy transposes during input prep; DMA transpose on TRN2 is
2-byte-dtype-only). The projection output Q^T/K^T [f, n] is then exactly
the [d, n] layout the scores matmul wants for both operands.

Matmuls run as float32r (same fp32 bytes, faster PE mode: 1 cycle/row vs
2-4 for plain fp32). The two heads that share an f-tile occupy PE row
groups 0:64 / 64:128 via tile_position so their K=64 score matmuls overlap.

Softmax skips the max-subtraction (scores are ~N(0,1) after the 1/8 scale;
exp never overflows fp32) so the only per-element passes are:
  PE matmul -> PSUM, ACT exp (+free row-sum accumulator) -> SBUF,
  DVE per-row scale -> SBUF, DMA -> HBM.
"""

import numpy as np
from contextlib import ExitStack

import concourse.bacc as bacc
import concourse.mybir as mybir
import concourse.tile as tile

# bass_utils imports antenv.axon_hooks when BASS_TRACE is set in the
# environment; some images ship an antenv stub without that module. Register
# a no-op fallback so tracing degrades gracefully instead of crashing.
try:
    from antenv.axon_hooks import get_axon_ntff_profile_hook as _g  # noqa: F401
except Exception:
    import sys as _sys
    import types as _types

    _m = _types.ModuleType("antenv.axon_hooks")
    _state = {"h": None}
    _m.set_axon_ntff_profile_hook = lambda h: _state.__setitem__("h", h)
    _m.get_axon_ntff_profile_hook = lambda: _state["h"]
    _sys.modules["antenv.axon_hooks"] = _m
    try:
        import antenv as _antenv

        _antenv.axon_hooks = _m
    except Exception:
        pass

from concourse.bass_utils import run_bass_kernel_spmd

B = 8          # batches == cores
N = 1024       # tokens
E = 768        # embed dim
H = 12         # heads
HD = 64        # head dim
F = H * HD     # 768 features per projection (Q or K)
ET = E // 128  # 6 e-tiles
FT = F // 128  # 6 f-tiles (2 heads per f-tile)
QB = N // 128  # 8 query blocks
SCALE = HD ** -0.5

_cache = {}


def _build(use_f32r=True):
    f32 = mybir.dt.float32
    mm_dt = mybir.dt.float32r if use_f32r else f32
    nc = bacc.Bacc("TRN2", debug=False, num_devices=B)

    xT_d = nc.dram_tensor("xT", [E, N], f32, kind="ExternalInput")
    wT_d = nc.dram_tensor("wT", [E, 2 * F], f32, kind="ExternalInput")
    f16 = mybir.dt.float16
    out_d = nc.dram_tensor("out", [H, N, N], f16, kind="ExternalOutput")

    xT_src = xT_d.ap().rearrange("(t p) n -> t p n", p=128)       # [6,128,1024]
    wT_src = wT_d.ap().rearrange("(t p) f -> t p f", p=128)       # [6,128,1536]
    out_flat = out_d.ap().rearrange("h q n -> (h q) n")           # [12288,1024]

    def mm(out_ap, lhsT, rhs, **kw):
        nc.tensor.matmul(out_ap, lhsT, rhs, **kw)

    with ExitStack() as ctx:
        tc = ctx.enter_context(tile.TileContext(nc))
        statics = ctx.enter_context(tc.tile_pool(name="statics", bufs=1))
        work = ctx.enter_context(tc.tile_pool(name="work", bufs=8))
        small = ctx.enter_context(tc.tile_pool(name="small", bufs=8))
        pproj = ctx.enter_context(tc.tile_pool(name="pproj", bufs=2, space="PSUM"))
        pscore = ctx.enter_context(tc.tile_pool(name="pscore", bufs=3, space="PSUM"))

        xt = statics.tile([128, ET, N], mm_dt, tag="xt", name="xt")
        wt = statics.tile([128, ET, 2 * F], mm_dt, tag="wt", name="wt")
        qt = statics.tile([128, FT, N], mm_dt, tag="qt", name="qt")
        kt = statics.tile([128, FT, N], mm_dt, tag="kt", name="kt")

        # Preload the exp table set while input DMAs run: a dependency-free
        # dummy ACTIVATE at t=0 pulls the ~2.7us ACT_TABLE_LOAD off the
        # critical path of the first real exp.
        warm = small.tile([128, 1], f32, tag="sums", name="warm")
        nc.vector.memset(warm, 0.0)
        nc.scalar.activation(warm, warm, mybir.ActivationFunctionType.Exp)

        # Input loads, chunked per e-tile so the first projection matmuls can
        # start as soon as the first chunks land.
        # Single sync-ring FIFO, priority-ordered: x chunks and the W columns
        # for f-tiles 0-1 first (they gate projections 0-1), then the rest in
        # f-tile order. 512-col chunks keep DMA descriptor runs at 2KB.
        for ei in range(ET):
            nc.sync.dma_start(xt[:, ei, :], xT_src[ei].bitcast(mm_dt))
            nc.sync.dma_start(wt[:, ei, 0:256], wT_src[ei][:, 0:256].bitcast(mm_dt))
        for ei in range(ET):
            nc.sync.dma_start(wt[:, ei, 256:512], wT_src[ei][:, 256:512].bitcast(mm_dt))
        for fg in range(1, 3):
            c0, c1 = fg * 512, (fg + 1) * 512
            for ei in range(ET):
                nc.sync.dma_start(
                    wt[:, ei, c0:c1], wT_src[ei][:, c0:c1].bitcast(mm_dt)
                )

        def emit_proj(fi):
            # qT/kT tile fi = W^T-cols.T @ x^T, as four single-bank [128,512]
            # accumulation tiles so projection holds only 2 PSUM banks
            # (bufs=2 keeps copy-read and next-group matmul-write in
            # disjoint banks), freeing banks for deeper scores buffering.
            # K halves first: kt gates every scores rhs.
            for dst, foff, nh in (
                (kt, (2 * fi + 1) * 128, 0),
                (kt, (2 * fi + 1) * 128, 1),
                (qt, 2 * fi * 128, 0),
                (qt, 2 * fi * 128, 1),
            ):
                pt = pproj.tile([128, 512], f32, tag="proj",
                                name=f"pp{fi}_{foff}_{nh}")
                for ei in range(ET):
                    mm(
                        pt,
                        lhsT=wt[:, ei, foff:foff + 128],
                        rhs=xt[:, ei, nh * 512:(nh + 1) * 512],
                        start=(ei == 0),
                        stop=(ei == ET - 1),
                    )
                nc.vector.tensor_copy(dst[:, fi, nh * 512:(nh + 1) * 512], pt)

        def emit_attn(fi):
            # scores + softmax for the two heads in this f-tile. Head 2fi
            # lives in partitions 0:64, head 2fi+1 in 64:128 -> their K=64
            # matmuls target different PE row groups and run concurrently.
            for qb in range(QB):
                scores = [
                    pscore.tile([128, N], f32, tag="ps", name=f"ps{fi}_{qb}_{hh}")
                    for hh in range(2)
                ]
                for hh in range(2):
                    for nh in range(2):
                        lo, hi = hh * 64, hh * 64 + 64
                        mm(
                            scores[hh][:, nh * 512:(nh + 1) * 512],
                            lhsT=qt[lo:hi, fi, qb * 128:(qb + 1) * 128],
                            rhs=kt[lo:hi, fi, nh * 512:(nh + 1) * 512],
                            start=True,
                            stop=True,
                            tile_position=(hh * 64, 0),
                        )
                for hh in range(2):
                    h = 2 * fi + hh
                    ot = work.tile([128, N], f16, tag="out", name=f"ot{fi}_{qb}_{hh}")
                    sums = small.tile([128, 1], f32, tag="sums", name=f"sm{fi}_{qb}_{hh}")
                    nc.scalar.activation(
                        ot, scores[hh], mybir.ActivationFunctionType.Exp,
                        scale=SCALE, accum_out=sums,
                    )
                    rec = small.tile([128, 1], f32, tag="rec", name=f"rc{fi}_{qb}_{hh}")
                    nc.vector.reciprocal(rec, sums)
                    nc.vector.tensor_scalar_mul(ot, ot, rec)
                    nc.sync.dma_start(
                        out_flat[h * N + qb * 128:h * N + (qb + 1) * 128], ot
                    )

        for fi in range(FT):
            emit_proj(fi)
            emit_attn(fi)

    nc.compile()
    return nc


def _run(x, W_qkv, trace=False, use_f32r=True):
    key = ("nc", use_f32r)
    if key not in _cache:
        _cache[key] = _build(use_f32r)
    nc = _cache[key]

    x = np.asarray(x, dtype=np.float32)
    W_qkv = np.asarray(W_qkv, dtype=np.float32)
    # interleave Q/K 128-col blocks per f-tile: [Q0,K0,Q1,K1,...,Q5,K5]
    wqk = W_qkv[: 2 * F].reshape(2, FT, 128, E)           # [qk, fi, 128, e]
    wqk = wqk.transpose(3, 1, 0, 2).reshape(E, 2 * F)     # [e, fi*qk*128]
    wT = np.ascontiguousarray(wqk)                        # [768, 1536]
    in_maps = [
        {"xT": np.ascontiguousarray(x[b].T), "wT": wT}
        for b in range(B)
    ]
    res = run_bass_kernel_spmd(nc, in_maps, core_ids=list(range(B)), trace=trace)
    out = np.stack([np.asarray(r["out"], dtype=np.float32) for r in res.results], axis=0)
    return out, res


def kernel(x, W_qkv):
    return _run(x, W_qkv)[0]



# revision 24
# speedup vs baseline: 4.4542x; 1.0667x over previous
"""Trainium2 Bass kernel for nn_Attention_layer_67877663146058.

Computes attn = softmax((x @ W_qkv.T)[q] @ (x @ W_qkv.T)[k]^T * hd**-0.5)
for x [8, 1024, 768], W_qkv [2304, 768] -> out [8, 12, 1024, 1024] fp32.

Sharding: batch-parallel across the 8 NeuronCores (core b handles batch b,
all 12 heads). The V third of the QKV projection never reaches the output,
so only the Q and K rows of W_qkv are used.

Layout strategy: the PE contracts over the partition dim of both operands,
so the projection needs x^T [e, n] and W^T [e, f] — both produced on the
host (cheap numpy transposes during input prep; DMA transpose on TRN2 is
2-byte-dtype-only). The projection output Q^T/K^T [f, n] is then exactly
the [d, n] layout the scores matmul wants for both operands.

Matmuls run as float32r (same fp32 bytes, faster PE mode: 1 cycle/row vs
2-4 for plain fp32). The two heads that share an f-tile occupy PE row
groups 0:64 / 64:128 via tile_position so their K=64 score matmuls overlap.

Softmax skips the max-subtraction (scores are ~N(0,1) after the 1/8 scale;
exp never overflows fp32) so the only per-element passes are:
  PE matmul -> PSUM, ACT exp (+free row-sum accumulator) -> SBUF,
  DVE per-row scale -> SBUF, DMA -> HBM.
"""

import numpy as np
from contextlib import ExitStack

import concourse.bacc as bacc
import concourse.mybir as mybir
import concourse.tile as tile

# bass_utils imports antenv.axon_hooks when BASS_TRACE is set in the
# environment; some images ship an antenv stub without that module. Register
# a no-op fallback so tracing degrades gracefully instead of crashing.
try:
    from antenv.axon_hooks import get_axon_ntff_profile_hook as _g  # noqa: F401
except Exception:
    import sys as _sys
    import types as _types

    _m = _types.ModuleType("antenv.axon_hooks")
    _state = {"h": None}
    _m.set_axon_ntff_profile_hook = lambda h: _state.__setitem__("h", h)
    _m.get_axon_ntff_profile_hook = lambda: _state["h"]
    _sys.modules["antenv.axon_hooks"] = _m
    try:
        import antenv as _antenv

        _antenv.axon_hooks = _m
    except Exception:
        pass

from concourse.bass_utils import run_bass_kernel_spmd

B = 8          # batches == cores
N = 1024       # tokens
E = 768        # embed dim
H = 12         # heads
HD = 64        # head dim
F = H * HD     # 768 features per projection (Q or K)
ET = E // 128  # 6 e-tiles
FT = F // 128  # 6 f-tiles (2 heads per f-tile)
QB = N // 128  # 8 query blocks
SCALE = HD ** -0.5

_cache = {}


def _build(use_f32r=True):
    f32 = mybir.dt.float32
    mm_dt = mybir.dt.float32r if use_f32r else f32
    nc = bacc.Bacc("TRN2", debug=False, num_devices=B)

    f16 = mybir.dt.float16
    xT_d = nc.dram_tensor("xT", [E, N], f16, kind="ExternalInput")
    wT_d = nc.dram_tensor("wT", [E, 2 * F], f16, kind="ExternalInput")
    out_d = nc.dram_tensor("out", [H, N, N], f16, kind="ExternalOutput")

    xT_src = xT_d.ap().rearrange("(t p) n -> t p n", p=128)       # [6,128,1024]
    wT_src = wT_d.ap().rearrange("(t p) f -> t p f", p=128)       # [6,128,1536]
    out_flat = out_d.ap().rearrange("h q n -> (h q) n")           # [12288,1024]

    def mm(out_ap, lhsT, rhs, **kw):
        nc.tensor.matmul(out_ap, lhsT, rhs, **kw)

    with ExitStack() as ctx:
        tc = ctx.enter_context(tile.TileContext(nc))
        statics = ctx.enter_context(tc.tile_pool(name="statics", bufs=1))
        work = ctx.enter_context(tc.tile_pool(name="work", bufs=8))
        small = ctx.enter_context(tc.tile_pool(name="small", bufs=8))
        pproj = ctx.enter_context(tc.tile_pool(name="pproj", bufs=2, space="PSUM"))
        pscore = ctx.enter_context(tc.tile_pool(name="pscore", bufs=3, space="PSUM"))

        # x/W arrive fp16 (half the input DMA bytes; fp16 matmuls run the
        # same 1 cycle/row as f32r). Q/K stay f32r: the PSUM->SBUF copies
        # are fp32 either way and scores matmuls are full speed.
        xt = statics.tile([128, ET, N], f16, tag="xt", name="xt")
        wt = statics.tile([128, ET, 2 * F], f16, tag="wt", name="wt")
        qt = statics.tile([128, FT, N], mm_dt, tag="qt", name="qt")
        kt = statics.tile([128, FT, N], mm_dt, tag="kt", name="kt")

        # Preload the exp table set while input DMAs run: a dependency-free
        # dummy ACTIVATE at t=0 pulls the ~2.7us ACT_TABLE_LOAD off the
        # critical path of the first real exp.
        warm = small.tile([128, 1], f32, tag="sums", name="warm")
        nc.vector.memset(warm, 0.0)
        nc.scalar.activation(warm, warm, mybir.ActivationFunctionType.Exp)

        # Input loads, chunked per e-tile so the first projection matmuls can
        # start as soon as the first chunks land.
        # Single sync-ring FIFO, priority-ordered: x chunks and the W columns
        # for f-tiles 0-1 first (they gate projections 0-1), then the rest in
        # f-tile order. 512-col chunks keep DMA descriptor runs at 2KB.
        for ei in range(ET):
            nc.sync.dma_start(xt[:, ei, :], xT_src[ei])
            nc.sync.dma_start(wt[:, ei, 0:256], wT_src[ei][:, 0:256])
        for ei in range(ET):
            nc.sync.dma_start(wt[:, ei, 256:512], wT_src[ei][:, 256:512])
        for fg in range(1, 3):
            c0, c1 = fg * 512, (fg + 1) * 512
            for ei in range(ET):
                nc.sync.dma_start(
                    wt[:, ei, c0:c1], wT_src[ei][:, c0:c1]
                )

        # Projection group g of f-tile fi: one [128,512] PSUM accumulator
        # (Q or K, one n-half), 6 accumulating matmuls + 1 DVE copy.
        # K halves first: kt gates every scores rhs.
        PROJ_GROUPS = (
            lambda fi: (kt, (2 * fi + 1) * 128, 0),
            lambda fi: (kt, (2 * fi + 1) * 128, 1),
            lambda fi: (qt, 2 * fi * 128, 0),
            lambda fi: (qt, 2 * fi * 128, 1),
        )

        def emit_proj_group(fi, g):
            dst, foff, nh = PROJ_GROUPS[g](fi)
            pt = pproj.tile([128, 512], f32, tag="proj",
                            name=f"pp{fi}_{foff}_{nh}")
            for ei in range(ET):
                mm(
                    pt,
                    lhsT=wt[:, ei, foff:foff + 128],
                    rhs=xt[:, ei, nh * 512:(nh + 1) * 512],
                    start=(ei == 0),
                    stop=(ei == ET - 1),
                )
            nc.vector.tensor_copy(dst[:, fi, nh * 512:(nh + 1) * 512], pt)

        def emit_attn_tile(fi, qb):
            # scores + softmax for the two heads of f-tile fi, one q-block.
            # Head 2fi lives in partitions 0:64, head 2fi+1 in 64:128 ->
            # their K=64 matmuls target different PE row groups.
            scores = [
                pscore.tile([128, N], f32, tag="ps", name=f"ps{fi}_{qb}_{hh}")
                for hh in range(2)
            ]
            for hh in range(2):
                for nh in range(2):
                    lo, hi = hh * 64, hh * 64 + 64
                    mm(
                        scores[hh][:, nh * 512:(nh + 1) * 512],
                        lhsT=qt[lo:hi, fi, qb * 128:(qb + 1) * 128],
                        rhs=kt[lo:hi, fi, nh * 512:(nh + 1) * 512],
                        start=True,
                        stop=True,
                        tile_position=(hh * 64, 0),
                    )
            for hh in range(2):
                h = 2 * fi + hh
                ot = work.tile([128, N], f16, tag="out", name=f"ot{fi}_{qb}_{hh}")
                sums = small.tile([128, 1], f32, tag="sums", name=f"sm{fi}_{qb}_{hh}")
                nc.scalar.activation(
                    ot, scores[hh], mybir.ActivationFunctionType.Exp,
                    scale=SCALE, accum_out=sums,
                )
                rec = small.tile([128, 1], f32, tag="rec", name=f"rc{fi}_{qb}_{hh}")
                nc.vector.reciprocal(rec, sums)
                nc.vector.tensor_scalar_mul(ot, ot, rec)
                nc.sync.dma_start(
                    out_flat[h * N + qb * 128:h * N + (qb + 1) * 128], ot
                )

        # Interleave: spread the next f-tile's four projection groups
        # between this f-tile's score tiles, so the in-order PE stream has
        # filler matmuls at every PSUM-stall point (gaps reset the tensor
        # engine's p-state ramp; continuous work lets it reach full clock).
        for g in range(4):
            emit_proj_group(0, g)
        for fi in range(FT):
            for qb in range(QB):
                emit_attn_tile(fi, qb)
                if fi + 1 < FT and qb % 2 == 0:
                    emit_proj_group(fi + 1, qb // 2)

    nc.compile()
    return nc


def _run(x, W_qkv, trace=False, use_f32r=True):
    key = ("nc", use_f32r)
    if key not in _cache:
        _cache[key] = _build(use_f32r)
    nc = _cache[key]

    x = np.asarray(x, dtype=np.float32)
    W_qkv = np.asarray(W_qkv, dtype=np.float32)
    # interleave Q/K 128-col blocks per f-tile: [Q0,K0,Q1,K1,...,Q5,K5]
    wqk = W_qkv[: 2 * F].reshape(2, FT, 128, E)           # [qk, fi, 128, e]
    wqk = wqk.transpose(3, 1, 0, 2).reshape(E, 2 * F)     # [e, fi*qk*128]
    wT = np.ascontiguousarray(wqk.astype(np.float16))     # [768, 1536]
    in_maps = [
        {"xT": np.ascontiguousarray(x[b].T.astype(np.float16)), "wT": wT}
        for b in range(B)
    ]
    res = run_bass_kernel_spmd(nc, in_maps, core_ids=list(range(B)), trace=trace)
    out = np.stack([np.asarray(r["out"], dtype=np.float32) for r in res.results], axis=0)
    return out, res


def kernel(x, W_qkv):
    return _run(x, W_qkv)[0]

